# revision 13
# baseline (speedup 1.0000x reference)
"""Bass/Trainium2 kernel for the edge-aware smoothness loss:

    sum over pixels of |grad log tmap|^2 * sigmoid(48*(0.1 - |grad mean(l_img)|))

Full inputs are sharded by rows across 8 NeuronCores (512 rows each).
Each core computes a partial sum over its interior rows; the 16 core-edge
rows (2 per core) are computed exactly on the host in float64 and added.

Per-core layout: partition p holds 4 consecutive rows (4p..4p+3) as 4
"segments" in the free dimension; W is processed in 8 chunks of 512 columns
(+1 halo column each side). Vertical stencil rows that cross partitions are
produced on the TensorEngine with +/-1 shift matrices; horizontal stencil and
in-partition vertical rows are also TensorEngine ident/shift matmul pairs
accumulated in PSUM. Squares/sums/reductions run on DVE custom fused ops,
transcendentals on ScalarE (table sets phased: Square -> Sqrt -> Sigmoid ->
Ln so each ACT table loads once).
"""

import sys

sys.path.insert(0, "/opt/trn_rl_repo")

import numpy as np

import concourse.bacc as bacc
import concourse.mybir as mybir
from concourse import bass_utils
from concourse import dve_ops
from concourse.dve_spec import Spec, Src0, Src1, C0, lower, sq, _has_src1
from concourse.dve_uop import DveOpSpec
from concourse.tile import TileContext
from concourse.tile_rust import add_dep_helper

EPS = 1e-07
SIG_OFFSET = 0.1
SIG_SCALE = 48.0

H, W = 4096, 4096
NCORES = 8
ROWS = H // NCORES          # 512 rows per core
S = 4                       # rows folded per partition
P = 128                     # partitions
NCHUNK = 8
WC = W // NCHUNK            # 512 columns per chunk
GW = WC + 2                 # chunk width incl. 1-col halo each side

F32 = mybir.dt.float32


# --------------------------------------------------------------------------
# custom DVE ops
# --------------------------------------------------------------------------

def _make_op(name: str, spec: Spec, row: int) -> dve_ops.DveOp:
    shas = {}
    for ver in ("v3", "v4"):
        try:
            s = DveOpSpec(name=name, opcode=row, uops=lower(spec, ver=ver),
                          rd1_en=_has_src1(spec))
            shas[ver] = s.sha(ver)
        except Exception:
            pass
    return dve_ops.DveOp(name, spec, subdim=False, uops_sha=shas)


def _register_custom_ops():
    if "ADDSQ_ANT" in dve_ops._SUB_OPCODE_FOR_NAME:
        return

    addsq_spec = Spec(
        body=Src0 + sq(Src1),
        reference=lambda in0, in1, s0, s1, imm2: (
            in0.astype(np.float32) + in1.astype(np.float32) ** 2
        ),
    )

    def _sqmulred_ref(in0, in1, c0, c1, c2):
        b = (in0.astype(np.float32) ** 2 * in1).astype(np.float32)
        acc = np.asarray(c0, np.float32).reshape(-1, 1) + b.reshape(
            b.shape[0], -1
        ).sum(axis=-1, keepdims=True)
        return b, acc

    from operator import add

    sqmulred_spec = Spec(
        body=sq(Src0) * Src1,
        accum=add,
        accum_init=C0,
        reference=_sqmulred_ref,
    )

    base = max(dve_ops._SUB_OPCODE_FOR_NAME.values()) + 1
    for i, (name, spec) in enumerate(
        [("ADDSQ_ANT", addsq_spec), ("SQMULRED_ANT", sqmulred_spec)]
    ):
        row = base + i
        assert row < 0x20, "custom-DVE opcode rows exhausted"
        dve_ops._SUB_OPCODE_FOR_NAME[name] = row
        op = _make_op(name, spec, row)
        dve_ops.OPS.append(op)
        dve_ops.CUSTOM_DVE_SPECS[name] = spec
        globals()["_OP_" + name] = op


_register_custom_ops()
_ADDSQ = next(o for o in dve_ops.OPS if o.name == "ADDSQ_ANT")
_SQMULRED = next(o for o in dve_ops.OPS if o.name == "SQMULRED_ANT")


# --------------------------------------------------------------------------
# device kernel
# --------------------------------------------------------------------------

MAT_NAMES = ["I", "nI", "I0", "nI0", "I127", "nI127", "Sd", "nSu"]


def make_mats() -> np.ndarray:
    """[128, 8*128] f32 stationary matrices.

    matmul(out, lhsT, rhs): out[p, j] = sum_k lhsT[k, p] * rhs[k, j].
    Sd[k, p] = 1 iff k == p-1  (out[p] = rhs[p-1], out[0] = 0)
    Su[k, p] = 1 iff k == p+1  (out[p] = rhs[p+1], out[127] = 0)
    I0/I127 are identities with column 0 / 127 zeroed so the stencil output
    for global edge rows (row 0 -> partition 0 seg 0, row 511 -> partition
    127 seg 3) is exactly zero; the host adds those rows' contribution.
    """
    eye = np.eye(P, dtype=np.float32)
    i0 = eye.copy(); i0[:, 0] = 0.0
    i127 = eye.copy(); i127[:, 127] = 0.0
    sd = np.eye(P, k=1, dtype=np.float32)    # [k, p]: 1 at p = k+1 -> k = p-1
    su = np.eye(P, k=-1, dtype=np.float32)   # 1 at p = k-1 -> k = p+1
    return np.concatenate(
        [eye, -eye, i0, -i0, i127, -i127, sd, -su], axis=1
    ).copy()


def _chunk_cols(c: int):
    """global g-grid columns [c*WC-1, c*WC+WC+1) clipped to [0, W).

    Returns (lo_clipped, n_cols, dst_off) where dst_off is the write offset
    into the GW-wide tile (pad columns at the image edge stay zero)."""
    lo = c * WC - 1
    lo_c = max(lo, 0)
    hi_c = min(c * WC + WC + 1, W)
    return lo_c, hi_c - lo_c, lo_c - lo


def build_kernel():
    nc = bacc.Bacc("TRN2", num_devices=NCORES)

    tm = nc.dram_tensor("tm", [ROWS, W], F32, kind="ExternalInput")
    li = nc.dram_tensor("li", [ROWS, W, 3], F32, kind="ExternalInput")
    mats = nc.dram_tensor("mats", [P, len(MAT_NAMES) * P], F32,
                          kind="ExternalInput")
    out = nc.dram_tensor("out", [P, 2], F32, kind="ExternalOutput")

    tm_v = tm.ap().rearrange("(p s) w -> p s w", s=S)          # [128, 4, 4096]
    li_v = li.ap().rearrange("(p s) w c -> p s (w c)", s=S)    # [128, 4, 12288]

    with TileContext(nc) as tc:
        with (
            tc.tile_pool(name="const", bufs=1) as cpool,
            tc.tile_pool(name="mf", bufs=1) as mfpool,
            tc.tile_pool(name="work", bufs=2) as wpool,
            tc.tile_pool(name="psum", bufs=1, space="PSUM") as ppool,
        ):
            mats_sb = cpool.tile([P, len(MAT_NAMES) * P], F32)
            nc.sync.dma_start(out=mats_sb[:], in_=mats.ap())
            M = {n: mats_sb[:, i * P:(i + 1) * P]
                 for i, n in enumerate(MAT_NAMES)}

            acc = cpool.tile([P, 2], F32)
            nc.vector.memset(acc[:], 0.0)

            # per-partition scalar constants for activation biases
            cb = cpool.tile([P, 3], F32)
            nc.vector.memset(cb[:, 0:1], SIG_SCALE * SIG_OFFSET)
            nc.vector.memset(cb[:, 1:2], -EPS)
            nc.vector.memset(cb[:, 2:3], EPS)
            b_sig, b_neps, b_eps = cb[:, 0:1], cb[:, 1:2], cb[:, 2:3]

            # sigmoid-weight buffer for the whole core: [128, 8 chunks * 2048]
            mf = mfpool.tile([P, NCHUNK * S * WC], F32)

            def stencils(src, ps_x, ps_y):
                """dx and dy of `src` ([128, 4, GW], 1-col halo) into PSUM
                ([128, 4, WC] each). Global edge rows (p0 seg0, p127 seg3)
                come out exactly zero via the masked matrices."""
                # horizontal: out[p,s,j] = src[p,s,j+2] - src[p,s,j]
                xmat = {0: ("I0", "nI0"), 1: ("I", "nI"),
                        2: ("I", "nI"), 3: ("I127", "nI127")}
                for s in range(S):
                    a, b = xmat[s]
                    nc.tensor.matmul(ps_x[:, s, :], M[a], src[:, s, 2:GW],
                                     start=True, stop=False)
                    nc.tensor.matmul(ps_x[:, s, :], M[b], src[:, s, 0:WC],
                                     start=False, stop=True)
                # vertical rows 4p (seg0): src[p-1, seg3] - src[p, seg1]
                nc.tensor.matmul(ps_y[:, 0, :], M["Sd"], src[:, 3, 1:WC + 1],
                                 start=True, stop=False)
                nc.tensor.matmul(ps_y[:, 0, :], M["nI0"], src[:, 1, 1:WC + 1],
                                 start=False, stop=True)
                # rows 4p+1 / 4p+2: in-partition segment pairs
                for s in (1, 2):
                    nc.tensor.matmul(ps_y[:, s, :], M["I"], src[:, s - 1, 1:WC + 1],
                                     start=True, stop=False)
                    nc.tensor.matmul(ps_y[:, s, :], M["nI"], src[:, s + 1, 1:WC + 1],
                                     start=False, stop=True)
                # rows 4p+3 (seg3): src[p, seg2] - src[p+1, seg0]
                nc.tensor.matmul(ps_y[:, 3, :], M["I127"], src[:, 2, 1:WC + 1],
                                 start=True, stop=False)
                nc.tensor.matmul(ps_y[:, 3, :], M["nSu"], src[:, 0, 1:WC + 1],
                                 start=False, stop=True)

            # ---------------- phase A: l_img -> sigmoid weights ----------
            li_dma_insts = []
            for c in range(NCHUNK):
                lo_c, ncols, off = _chunk_cols(c)
                li_t = wpool.tile([P, S, 3 * GW], F32, tag="li")
                d = nc.sync.dma_start(
                    out=li_t[:, :, 3 * off:3 * (off + ncols)],
                    in_=li_v[:, :, 3 * lo_c:3 * (lo_c + ncols)],
                )
                li_dma_insts.append(d)

                g_t = wpool.tile([P, S, GW], F32, tag="g")
                if off:
                    nc.vector.memset(g_t[:, :, 0:off], 0.0)
                if off + ncols < GW:
                    nc.vector.memset(g_t[:, :, off + ncols:GW], 0.0)
                nc.vector.reduce_sum(
                    out=g_t[:, :, off:off + ncols],
                    in_=li_t[:, :, 3 * off:3 * (off + ncols)].rearrange(
                        "p s (w c) -> p s w c", c=3
                    ),
                    axis=mybir.AxisListType.X,
                )

                ps_x = ppool.tile([P, S, WC], F32, tag="psx")
                ps_y = ppool.tile([P, S, WC], F32, tag="psy")
                stencils(g_t, ps_x, ps_y)

                a_t = wpool.tile([P, S * WC], F32, tag="a")
                nc.scalar.square(out=a_t[:], in_=ps_x[:].rearrange("p s j -> p (s j)"))

                # m = dgx^2 + dgy^2 (scaled by 1/9 later inside Sqrt)
                nc.vector._custom_dve(
                    _ADDSQ,
                    out=mf[:, c * S * WC:(c + 1) * S * WC],
                    in0=a_t[:],
                    in1=ps_y[:].rearrange("p s j -> p (s j)"),
                )

            # ---------------- phase A2/A3: sqrt then sigmoid -------------
            nc.scalar.activation(out=mf[:], in_=mf[:],
                                 func=mybir.ActivationFunctionType.Sqrt,
                                 scale=1.0 / 9.0)
            sig_i = nc.scalar.activation(out=mf[:], in_=mf[:],
                                         func=mybir.ActivationFunctionType.Sigmoid,
                                         scale=-SIG_SCALE,
                                         bias=b_sig)
            # global edge rows (row 0 / row 511) contribute zero on device:
            # the masked matrices zero their dlx/dly PSUM rows in phase B,
            # so n*sig = 0 there; the host adds their exact contribution.

            # ---------------- phase B: tmap -> log grad, reduce ----------
            for c in range(NCHUNK):
                lo_c, ncols, off = _chunk_cols(c)
                tm_t = wpool.tile([P, S, GW], F32, tag="tm")
                d = nc.sync.dma_start(
                    out=tm_t[:, :, off:off + ncols],
                    in_=tm_v[:, :, lo_c:lo_c + ncols],
                )
                # keep the DMA rings draining phase-A loads first
                add_dep_helper(d.ins, li_dma_insts[-1].ins, sync=False,
                               reason="phase A dma first")

                # lg = ln(max(tm, eps)) in place; pad cols = 0 (log-pad)
                live = tm_t[:, :, off:off + ncols]
                nc.scalar.activation(out=live, in_=live,
                                     func=mybir.ActivationFunctionType.Relu,
                                     bias=b_neps)
                ln_i = nc.scalar.activation(out=live, in_=live,
                                            func=mybir.ActivationFunctionType.Ln,
                                            bias=b_eps)
                add_dep_helper(ln_i.ins, sig_i.ins, sync=False,
                               reason="act table phase order")
                if off:
                    nc.vector.memset(tm_t[:, :, 0:off], 0.0)
                if off + ncols < GW:
                    nc.vector.memset(tm_t[:, :, off + ncols:GW], 0.0)

                ps_x = ppool.tile([P, S, WC], F32, tag="psx")
                ps_y = ppool.tile([P, S, WC], F32, tag="psy")
                stencils(tm_t, ps_x, ps_y)

                scr = wpool.tile([P, S * WC], F32, tag="scr")
                sig_c = mf[:, c * S * WC:(c + 1) * S * WC]
                nc.vector._custom_dve(
                    _SQMULRED,
                    out=scr[:],
                    in0=ps_x[:].rearrange("p s j -> p (s j)"),
                    in1=sig_c,
                    s0=acc[:, 0:1],
                    accum_out=acc[:, 0:1],
                )
                scr2 = wpool.tile([P, S * WC], F32, tag="scr2")
                nc.vector._custom_dve(
                    _SQMULRED,
                    out=scr2[:],
                    in0=ps_y[:].rearrange("p s j -> p (s j)"),
                    in1=sig_c,
                    s0=acc[:, 1:2],
                    accum_out=acc[:, 1:2],
                )

            nc.sync.dma_start(out=out.ap(), in_=acc[:])

    nc.finalize()
    return nc


_NC_CACHE = None


def _get_nc():
    global _NC_CACHE
    if _NC_CACHE is None:
        _NC_CACHE = build_kernel()
    return _NC_CACHE


# --------------------------------------------------------------------------
# host-side edge rows (exact, float64)
# --------------------------------------------------------------------------

def _edge_contribution(tmap: np.ndarray, l_img: np.ndarray) -> float:
    """Exact contribution of global rows {512c, 512c+511} in float64."""
    rows = []
    for c in range(NCORES):
        rows.append(c * ROWS)
        rows.append(c * ROWS + ROWS - 1)

    logp = np.log(np.clip(tmap.astype(np.float64), EPS, 1.0))
    g = l_img.astype(np.float64).mean(axis=2)

    def pad_row(a, r):
        return a[r] if 0 <= r < H else np.zeros(W, np.float64)

    total = 0.0
    for r in rows:
        lc, lu, ld = logp[r], pad_row(logp, r - 1), pad_row(logp, r + 1)
        gc, gu, gd = g[r], pad_row(g, r - 1), pad_row(g, r + 1)
        zl = np.zeros(1, np.float64)

        def dx(v):
            return np.concatenate([v[1:], zl]) - np.concatenate([zl, v[:-1]])

        n = dx(lc) ** 2 + (lu - ld) ** 2
        s = np.sqrt(dx(gc) ** 2 + (gu - gd) ** 2)
        sig = 1.0 / (1.0 + np.exp(-(SIG_OFFSET - s) * SIG_SCALE))
        total += float(np.sum(n * sig))
    return total


# --------------------------------------------------------------------------
# entry point
# --------------------------------------------------------------------------

def run_device(tmap: np.ndarray, l_img: np.ndarray, **kw):
    nc = _get_nc()
    mats = make_mats()
    in_maps = [
        {
            "tm": np.ascontiguousarray(tmap[c * ROWS:(c + 1) * ROWS]),
            "li": np.ascontiguousarray(l_img[c * ROWS:(c + 1) * ROWS]),
            "mats": mats,
        }
        for c in range(NCORES)
    ]
    return bass_utils.run_bass_kernel_spmd(
        nc, in_maps, core_ids=list(range(NCORES)), **kw
    )


def kernel(tmap: np.ndarray, l_img: np.ndarray) -> np.ndarray:
    res = run_device(tmap, l_img)
    dev = sum(float(r["out"].astype(np.float64).sum()) for r in res.results)
    return np.float32(dev + _edge_contribution(tmap, l_img))


if __name__ == "__main__":
    tmap = np.random.rand(H, W).astype(np.float32)
    l_img = np.random.rand(H, W, 3).astype(np.float32)
    print(kernel(tmap, l_img))


# revision 14
# speedup vs baseline: 1.5873x; 1.5873x over previous
"""Bass/Trainium2 kernel for the edge-aware smoothness loss:

    sum over pixels of |grad log tmap|^2 * sigmoid(48*(0.1 - |grad mean(l_img)|))

Full inputs are sharded by rows across 8 NeuronCores (512 rows each).
Each core computes a partial sum over its interior rows; the 16 core-edge
rows (2 per core) are computed exactly on the host in float64 and added.

Per-core layout: partition p holds 4 consecutive rows (4p..4p+3) as 4
"segments" in the free dimension; W is processed in 8 chunks of 512 columns
(+1 halo column each side). Vertical stencil rows that cross partitions are
produced on the TensorEngine with +/-1 shift matrices; horizontal stencil and
in-partition vertical rows are also TensorEngine ident/shift matmul pairs
accumulated in PSUM. Squares/sums/reductions run on DVE custom fused ops,
transcendentals on ScalarE (table sets phased: Square -> Sqrt -> Sigmoid ->
Ln so each ACT table loads once).
"""

import sys

sys.path.insert(0, "/opt/trn_rl_repo")

import numpy as np

import concourse.bacc as bacc
import concourse.mybir as mybir
from concourse import bass_utils
from concourse import dve_ops
from concourse.dve_spec import Spec, Src0, Src1, C0, lower, sq, _has_src1
from concourse.dve_uop import DveOpSpec
from concourse.tile import TileContext
from concourse.tile_rust import add_dep_helper

EPS = 1e-07
SIG_OFFSET = 0.1
SIG_SCALE = 48.0

H, W = 4096, 4096
NCORES = 8
ROWS = H // NCORES          # 512 rows per core
S = 4                       # rows folded per partition
P = 128                     # partitions
NCHUNK = 8
WC = W // NCHUNK            # 512 columns per chunk
GW = WC + 2                 # chunk width incl. 1-col halo each side

F32 = mybir.dt.float32
BF16 = mybir.dt.bfloat16


# --------------------------------------------------------------------------
# custom DVE ops
# --------------------------------------------------------------------------

def _make_op(name: str, spec: Spec, row: int) -> dve_ops.DveOp:
    shas = {}
    for ver in ("v3", "v4"):
        try:
            s = DveOpSpec(name=name, opcode=row, uops=lower(spec, ver=ver),
                          rd1_en=_has_src1(spec))
            shas[ver] = s.sha(ver)
        except Exception:
            pass
    return dve_ops.DveOp(name, spec, subdim=False, uops_sha=shas)


def _register_custom_ops():
    if "ADDSQ_ANT" in dve_ops._SUB_OPCODE_FOR_NAME:
        return

    addsq_spec = Spec(
        body=Src0 + sq(Src1),
        reference=lambda in0, in1, s0, s1, imm2: (
            in0.astype(np.float32) + in1.astype(np.float32) ** 2
        ),
    )

    def _sqmulred_ref(in0, in1, c0, c1, c2):
        b = (in0.astype(np.float32) ** 2 * in1).astype(np.float32)
        acc = np.asarray(c0, np.float32).reshape(-1, 1) + b.reshape(
            b.shape[0], -1
        ).sum(axis=-1, keepdims=True)
        return b, acc

    from operator import add

    sqmulred_spec = Spec(
        body=sq(Src0) * Src1,
        accum=add,
        accum_init=C0,
        reference=_sqmulred_ref,
    )

    base = max(dve_ops._SUB_OPCODE_FOR_NAME.values()) + 1
    for i, (name, spec) in enumerate(
        [("ADDSQ_ANT", addsq_spec), ("SQMULRED_ANT", sqmulred_spec)]
    ):
        row = base + i
        assert row < 0x20, "custom-DVE opcode rows exhausted"
        dve_ops._SUB_OPCODE_FOR_NAME[name] = row
        op = _make_op(name, spec, row)
        dve_ops.OPS.append(op)
        dve_ops.CUSTOM_DVE_SPECS[name] = spec
        globals()["_OP_" + name] = op


_register_custom_ops()
_ADDSQ = next(o for o in dve_ops.OPS if o.name == "ADDSQ_ANT")
_SQMULRED = next(o for o in dve_ops.OPS if o.name == "SQMULRED_ANT")


# --------------------------------------------------------------------------
# device kernel
# --------------------------------------------------------------------------

MAT_NAMES = ["I", "nI", "I0", "nI0", "I127", "nI127", "Sd", "nSu"]


def make_mats() -> np.ndarray:
    """[128, 8*128] f32 stationary matrices.

    matmul(out, lhsT, rhs): out[p, j] = sum_k lhsT[k, p] * rhs[k, j].
    Sd[k, p] = 1 iff k == p-1  (out[p] = rhs[p-1], out[0] = 0)
    Su[k, p] = 1 iff k == p+1  (out[p] = rhs[p+1], out[127] = 0)
    I0/I127 are identities with column 0 / 127 zeroed so the stencil output
    for global edge rows (row 0 -> partition 0 seg 0, row 511 -> partition
    127 seg 3) is exactly zero; the host adds those rows' contribution.
    """
    eye = np.eye(P, dtype=np.float32)
    i0 = eye.copy(); i0[:, 0] = 0.0
    i127 = eye.copy(); i127[:, 127] = 0.0
    sd = np.eye(P, k=1, dtype=np.float32)    # [k, p]: 1 at p = k+1 -> k = p-1
    su = np.eye(P, k=-1, dtype=np.float32)   # 1 at p = k-1 -> k = p+1
    import ml_dtypes
    return np.concatenate(
        [eye, -eye, i0, -i0, i127, -i127, sd, -su], axis=1
    ).astype(ml_dtypes.bfloat16).copy()


def _chunk_cols(c: int):
    """global g-grid columns [c*WC-1, c*WC+WC+1) clipped to [0, W).

    Returns (lo_clipped, n_cols, dst_off) where dst_off is the write offset
    into the GW-wide tile (pad columns at the image edge stay zero)."""
    lo = c * WC - 1
    lo_c = max(lo, 0)
    hi_c = min(c * WC + WC + 1, W)
    return lo_c, hi_c - lo_c, lo_c - lo


def build_kernel():
    nc = bacc.Bacc("TRN2", num_devices=NCORES)

    tm = nc.dram_tensor("tm", [ROWS, W], F32, kind="ExternalInput")
    li = nc.dram_tensor("li", [ROWS, W, 3], F32, kind="ExternalInput")
    mats = nc.dram_tensor("mats", [P, len(MAT_NAMES) * P], BF16,
                          kind="ExternalInput")
    out = nc.dram_tensor("out", [P, 2], F32, kind="ExternalOutput")

    tm_v = tm.ap().rearrange("(p s) w -> p s w", s=S)          # [128, 4, 4096]
    li_v = li.ap().rearrange("(p s) w c -> p s (w c)", s=S)    # [128, 4, 12288]

    with TileContext(nc) as tc:
        with (
            tc.tile_pool(name="const", bufs=1) as cpool,
            tc.tile_pool(name="mf", bufs=1) as mfpool,
            tc.tile_pool(name="work", bufs=2) as wpool,
            tc.tile_pool(name="psum", bufs=1, space="PSUM") as ppool,
        ):
            mats_sb = cpool.tile([P, len(MAT_NAMES) * P], BF16)
            nc.sync.dma_start(out=mats_sb[:], in_=mats.ap())
            M = {n: mats_sb[:, i * P:(i + 1) * P]
                 for i, n in enumerate(MAT_NAMES)}

            acc = cpool.tile([P, 2], F32)
            nc.vector.memset(acc[:], 0.0)

            # per-partition scalar constants for activation biases
            cb = cpool.tile([P, 3], F32)
            nc.vector.memset(cb[:, 0:1], SIG_SCALE * SIG_OFFSET)
            nc.vector.memset(cb[:, 1:2], -EPS)
            nc.vector.memset(cb[:, 2:3], EPS)
            b_sig, b_neps, b_eps = cb[:, 0:1], cb[:, 1:2], cb[:, 2:3]

            # sigmoid-weight buffer for the whole core: [128, 8 chunks * 2048]
            mf = mfpool.tile([P, NCHUNK * S * WC], F32)

            def stencils(src, ps_x, ps_y):
                """dx and dy of `src` ([128, 4, GW], 1-col halo) into PSUM
                ([128, 4, WC] each). Global edge rows (p0 seg0, p127 seg3)
                come out exactly zero via the masked matrices."""
                # horizontal: out[p,s,j] = src[p,s,j+2] - src[p,s,j]
                xmat = {0: ("I0", "nI0"), 1: ("I", "nI"),
                        2: ("I", "nI"), 3: ("I127", "nI127")}
                for s in range(S):
                    a, b = xmat[s]
                    nc.tensor.matmul(ps_x[:, s, :], M[a], src[:, s, 2:GW],
                                     start=True, stop=False)
                    nc.tensor.matmul(ps_x[:, s, :], M[b], src[:, s, 0:WC],
                                     start=False, stop=True)
                # vertical rows 4p (seg0): src[p-1, seg3] - src[p, seg1]
                nc.tensor.matmul(ps_y[:, 0, :], M["Sd"], src[:, 3, 1:WC + 1],
                                 start=True, stop=False)
                nc.tensor.matmul(ps_y[:, 0, :], M["nI0"], src[:, 1, 1:WC + 1],
                                 start=False, stop=True)
                # rows 4p+1 / 4p+2: in-partition segment pairs
                for s in (1, 2):
                    nc.tensor.matmul(ps_y[:, s, :], M["I"], src[:, s - 1, 1:WC + 1],
                                     start=True, stop=False)
                    nc.tensor.matmul(ps_y[:, s, :], M["nI"], src[:, s + 1, 1:WC + 1],
                                     start=False, stop=True)
                # rows 4p+3 (seg3): src[p, seg2] - src[p+1, seg0]
                nc.tensor.matmul(ps_y[:, 3, :], M["I127"], src[:, 2, 1:WC + 1],
                                 start=True, stop=False)
                nc.tensor.matmul(ps_y[:, 3, :], M["nSu"], src[:, 0, 1:WC + 1],
                                 start=False, stop=True)

            # ---------------- phase A: l_img -> sigmoid weights ----------
            li_dma_insts = []
            for c in range(NCHUNK):
                lo_c, ncols, off = _chunk_cols(c)
                li_t = wpool.tile([P, S, 3 * GW], F32, tag="li")
                d = nc.sync.dma_start(
                    out=li_t[:, :, 3 * off:3 * (off + ncols)],
                    in_=li_v[:, :, 3 * lo_c:3 * (lo_c + ncols)],
                )
                li_dma_insts.append(d)

                g_t = wpool.tile([P, S, GW], BF16, tag="g")
                if off:
                    nc.vector.memset(g_t[:, :, 0:off], 0.0)
                if off + ncols < GW:
                    nc.vector.memset(g_t[:, :, off + ncols:GW], 0.0)
                with nc.allow_low_precision("bf16 stencil inputs"):
                    nc.vector.reduce_sum(
                        out=g_t[:, :, off:off + ncols],
                        in_=li_t[:, :, 3 * off:3 * (off + ncols)].rearrange(
                            "p s (w c) -> p s w c", c=3
                        ),
                        axis=mybir.AxisListType.X,
                    )

                ps_x = ppool.tile([P, S, WC], F32, tag="psx")
                ps_y = ppool.tile([P, S, WC], F32, tag="psy")
                stencils(g_t, ps_x, ps_y)

                a_t = wpool.tile([P, S * WC], F32, tag="a")
                nc.scalar.square(out=a_t[:], in_=ps_x[:].rearrange("p s j -> p (s j)"))

                # m = dgx^2 + dgy^2 (scaled by 1/9 later inside Sqrt)
                nc.vector._custom_dve(
                    _ADDSQ,
                    out=mf[:, c * S * WC:(c + 1) * S * WC],
                    in0=a_t[:],
                    in1=ps_y[:].rearrange("p s j -> p (s j)"),
                )

            # ---------------- phase A2/A3: sqrt then sigmoid -------------
            nc.scalar.activation(out=mf[:], in_=mf[:],
                                 func=mybir.ActivationFunctionType.Sqrt,
                                 scale=1.0 / 9.0)
            sig_i = nc.scalar.activation(out=mf[:], in_=mf[:],
                                         func=mybir.ActivationFunctionType.Sigmoid,
                                         scale=-SIG_SCALE,
                                         bias=b_sig)
            # global edge rows (row 0 / row 511) contribute zero on device:
            # the masked matrices zero their dlx/dly PSUM rows in phase B,
            # so n*sig = 0 there; the host adds their exact contribution.

            # ---------------- phase B: tmap -> log grad, reduce ----------
            for c in range(NCHUNK):
                lo_c, ncols, off = _chunk_cols(c)
                tm_t = wpool.tile([P, S, GW], F32, tag="tm")
                d = nc.sync.dma_start(
                    out=tm_t[:, :, off:off + ncols],
                    in_=tm_v[:, :, lo_c:lo_c + ncols],
                )
                # keep the DMA rings draining phase-A loads first
                add_dep_helper(d.ins, li_dma_insts[-1].ins, sync=False,
                               reason="phase A dma first")

                # lg = ln(max(tm, eps)) -> bf16; pad cols = 0 (log-pad)
                live = tm_t[:, :, off:off + ncols]
                nc.scalar.activation(out=live, in_=live,
                                     func=mybir.ActivationFunctionType.Relu,
                                     bias=b_neps)
                lg_t = wpool.tile([P, S, GW], BF16, tag="lg")
                ln_i = nc.scalar.activation(out=lg_t[:, :, off:off + ncols],
                                            in_=live,
                                            func=mybir.ActivationFunctionType.Ln,
                                            bias=b_eps)
                add_dep_helper(ln_i.ins, sig_i.ins, sync=False,
                               reason="act table phase order")
                if off:
                    nc.vector.memset(lg_t[:, :, 0:off], 0.0)
                if off + ncols < GW:
                    nc.vector.memset(lg_t[:, :, off + ncols:GW], 0.0)

                ps_x = ppool.tile([P, S, WC], F32, tag="psx")
                ps_y = ppool.tile([P, S, WC], F32, tag="psy")
                stencils(lg_t, ps_x, ps_y)

                scr = wpool.tile([P, S * WC], F32, tag="scr")
                sig_c = mf[:, c * S * WC:(c + 1) * S * WC]
                nc.vector._custom_dve(
                    _SQMULRED,
                    out=scr[:],
                    in0=ps_x[:].rearrange("p s j -> p (s j)"),
                    in1=sig_c,
                    s0=acc[:, 0:1],
                    accum_out=acc[:, 0:1],
                )
                scr2 = wpool.tile([P, S * WC], F32, tag="scr2")
                nc.vector._custom_dve(
                    _SQMULRED,
                    out=scr2[:],
                    in0=ps_y[:].rearrange("p s j -> p (s j)"),
                    in1=sig_c,
                    s0=acc[:, 1:2],
                    accum_out=acc[:, 1:2],
                )

            nc.sync.dma_start(out=out.ap(), in_=acc[:])

    nc.finalize()
    return nc


_NC_CACHE = None


def _get_nc():
    global _NC_CACHE
    if _NC_CACHE is None:
        _NC_CACHE = build_kernel()
    return _NC_CACHE


# --------------------------------------------------------------------------
# host-side edge rows (exact, float64)
# --------------------------------------------------------------------------

def _edge_contribution(tmap: np.ndarray, l_img: np.ndarray) -> float:
    """Exact contribution of global rows {512c, 512c+511} in float64."""
    rows = []
    for c in range(NCORES):
        rows.append(c * ROWS)
        rows.append(c * ROWS + ROWS - 1)

    logp = np.log(np.clip(tmap.astype(np.float64), EPS, 1.0))
    g = l_img.astype(np.float64).mean(axis=2)

    def pad_row(a, r):
        return a[r] if 0 <= r < H else np.zeros(W, np.float64)

    total = 0.0
    for r in rows:
        lc, lu, ld = logp[r], pad_row(logp, r - 1), pad_row(logp, r + 1)
        gc, gu, gd = g[r], pad_row(g, r - 1), pad_row(g, r + 1)
        zl = np.zeros(1, np.float64)

        def dx(v):
            return np.concatenate([v[1:], zl]) - np.concatenate([zl, v[:-1]])

        n = dx(lc) ** 2 + (lu - ld) ** 2
        s = np.sqrt(dx(gc) ** 2 + (gu - gd) ** 2)
        sig = 1.0 / (1.0 + np.exp(-(SIG_OFFSET - s) * SIG_SCALE))
        total += float(np.sum(n * sig))
    return total


# --------------------------------------------------------------------------
# entry point
# --------------------------------------------------------------------------

def run_device(tmap: np.ndarray, l_img: np.ndarray, **kw):
    nc = _get_nc()
    mats = make_mats()
    in_maps = [
        {
            "tm": np.ascontiguousarray(tmap[c * ROWS:(c + 1) * ROWS]),
            "li": np.ascontiguousarray(l_img[c * ROWS:(c + 1) * ROWS]),
            "mats": mats,
        }
        for c in range(NCORES)
    ]
    return bass_utils.run_bass_kernel_spmd(
        nc, in_maps, core_ids=list(range(NCORES)), **kw
    )


def kernel(tmap: np.ndarray, l_img: np.ndarray) -> np.ndarray:
    res = run_device(tmap, l_img)
    dev = sum(float(r["out"].astype(np.float64).sum()) for r in res.results)
    return np.float32(dev + _edge_contribution(tmap, l_img))


if __name__ == "__main__":
    tmap = np.random.rand(H, W).astype(np.float32)
    l_img = np.random.rand(H, W, 3).astype(np.float32)
    print(kernel(tmap, l_img))
